# revision 7
# baseline (speedup 1.0000x reference)
# Trainium2 Bass kernel for nn_Conv2dSDK_QR: low-rank (Q @ R) factorized
# stride-1 3x3 conv expressed as two matmuls over 4x4/stride-2 windows.
#
# Math (per image, validated vs reference):
#   xp = zero-pad(x, 1)                              [128, 66, 66]
#   flatT[win*128+c, vi*32+vj] = xp[c, i+2vi, j+2vj] (win = i*4+j)
#   tT = R2 @ flatT                                  [256, 1024]
#   yT = Q @ tT                                      [512, 1024]
#   out[oc, 2vi+top, 2vj+left] = yT[(top*2+left)*128+oc, vi*32+vj]
# where R2 is R with columns permuted from (c*16+win) to (win*128+c)
# ordering, so each win-chunk of flatT is just a strided view of xp.
#
# Device layouts (host pre/post-processed so every PE stream and every DMA
# is contiguous):
#   space-to-depth: s2d[c, pi, pj, hi, wi] = xp[c, 2hi+pi, 2wi+pj] (66=2x33)
#   x3[lb][c, pi, pj, h, w] = s2d[c, pi, pj, 16*lb + h, w], h in [0,17)
#     (l-block chunks, boundary row hi=16 duplicated) -> window (i,j) of
#     l-block lb is the contiguous-inner view
#     x3[lb][:, i&1, j&1, (i>>1):(i>>1)+16, (j>>1):(j>>1)+32]
#   y per l-block: y3[lb][oc, top, left, vi_in, vj] = out[oc, 2(16lb+vi_in)+top, 2vj+left]
#
# Sharding: data-parallel over batch, 4 images per core across 8 cores.

import numpy as np

import concourse.bacc as bacc
import concourse.bass as bass
import concourse.mybir as mybir
import concourse.tile as tile
from concourse.bass_utils import run_bass_kernel_spmd

N_CORES = 8
N_PER_CORE = 4
C = 128          # channels (= partition dim)
H = W = 64
RANK = 256
MOUT = 512       # 4 placements * 128 out channels
NWIN = 16        # 4x4 window positions
DT = mybir.dt.float32
MM_DT = mybir.dt.float32r


def build_nc(n_per_core=N_PER_CORE, mm_dtype=MM_DT):
    nc = bacc.Bacc()
    x_ext = nc.declare_dram_parameter("x", [n_per_core, 2, C, 2, 2, 17, 33], mm_dtype, isOutput=False)
    r_ext = nc.declare_dram_parameter("r2t", [2, C, NWIN, 128], mm_dtype, isOutput=False)
    q_ext = nc.declare_dram_parameter("qt", [C, 2, MOUT], mm_dtype, isOutput=False)
    y_ext = nc.declare_dram_parameter("y", [n_per_core, 2, C, 2, 2, 16, 32], DT, isOutput=True)

    with tile.TileContext(nc) as tc:
        with (
            tc.tile_pool(name="weights", bufs=1) as wpool,
            tc.tile_pool(name="xp", bufs=4) as xpool,
            tc.tile_pool(name="tt", bufs=2) as tpool,
            tc.tile_pool(name="osb", bufs=4) as opool,
            tc.tile_pool(name="pt", bufs=4, space="PSUM") as ptpool,
            tc.tile_pool(name="py", bufs=4, space="PSUM") as pypool,
        ):
            # r2t[rc][c, win, r_in] = R2[rc*128+r_in, win*128+c]; rc0 first —
            # it gates the first matmul group.
            r2t = [wpool.tile([C, NWIN, 128], mm_dtype, tag=f"r2t{rc}", name=f"r2t{rc}") for rc in range(2)]
            # qt[r_in, rc, m] = Q[m, rc*128+r_in]  (lhsT chunks for matmul 2)
            qt = wpool.tile([C, 2, MOUT], mm_dtype)
            nc.scalar.dma_start(r2t[0][:], r_ext[0])
            nc.scalar.dma_start(r2t[1][:], r_ext[1])
            nc.scalar.dma_start(qt[:], q_ext[:])

            for n in range(n_per_core):
                x3 = [xpool.tile([C, 2, 2, 17, 33], mm_dtype, tag="x3", name=f"x3_{n}_{lb}") for lb in range(2)]
                nc.sync.dma_start(x3[0][:], x_ext[n, 0])
                nc.sync.dma_start(x3[1][:], x_ext[n, 1])
                # tT[r_in, rc, vi_in, vj] per l-block
                tT = tpool.tile([C, 2, 2, 16, 32], mm_dtype)
                for lb in range(2):   # l-blocks of 512 positions (16 vi rows)
                    for rc in range(2):   # rank tiles of 128
                        pt = ptpool.tile([128, 16, 32], DT)
                        for win in range(NWIN):
                            i, j = divmod(win, 4)
                            rhs = x3[lb][:, i & 1, j & 1,
                                         (i >> 1) : (i >> 1) + 16,
                                         (j >> 1) : (j >> 1) + 32]
                            nc.tensor.matmul(
                                pt[:],
                                r2t[rc][:, win, :],
                                rhs,
                                start=(win == 0),
                                stop=(win == NWIN - 1),
                            )
                        nc.vector.tensor_copy(tT[:, rc, lb], pt[:])
                    osb = opool.tile([C, 2, 2, 16, 32], DT, tag="osb")
                    for mt in range(4):   # output row tiles: m = mt*128 + oc
                        py = pypool.tile([128, 16, 32], DT)
                        for rc in range(2):
                            nc.tensor.matmul(
                                py[:],
                                qt[:, rc, mt * 128 : (mt + 1) * 128],
                                tT[:, rc, lb],
                                start=(rc == 0),
                                stop=(rc == 1),
                            )
                        top, left = divmod(mt, 2)
                        nc.vector.tensor_copy(osb[:, top, left], py[:])
                    nc.scalar.dma_start(y_ext[n, lb], osb[:])
    nc.finalize()
    return nc


def make_host_inputs(x, Q, R, np_dtype=np.float32):
    """Full inputs -> (x3 chunks, r2t halves, qt) host arrays."""
    x = np.asarray(x, dtype=np.float32)
    Q = np.asarray(Q, dtype=np.float32)
    R = np.asarray(R, dtype=np.float32)
    n = x.shape[0]
    xpad = np.zeros((n, C, 66, 66), np.float32)
    xpad[:, :, 1 : 1 + H, 1 : 1 + W] = x
    # space-to-depth: s2d[n, c, pi, pj, hi, wi] = xpad[n, c, 2hi+pi, 2wi+pj]
    s2d = xpad.reshape(n, C, 33, 2, 33, 2).transpose(0, 1, 3, 5, 2, 4)
    # l-block chunks with duplicated boundary row hi=16:
    # x3[n, lb, c, pi, pj, h, w] = s2d[n, c, pi, pj, 16*lb+h, w]
    x3 = np.empty((n, 2, C, 2, 2, 17, 33), np.float32)
    x3[:, 0] = s2d[:, :, :, :, 0:17]
    x3[:, 1] = s2d[:, :, :, :, 16:33]
    x3 = np.ascontiguousarray(x3).astype(np_dtype)
    # permute R columns from (c*16+win) to (win*128+c), split by rank half
    R2 = R.reshape(RANK, C, NWIN).transpose(0, 2, 1).reshape(RANK, C * NWIN)
    r2t = R2.reshape(2, 128, NWIN, C).transpose(0, 3, 2, 1)  # [rc, c, win, r_in]
    r2t = np.ascontiguousarray(r2t).astype(np_dtype)
    qt = np.ascontiguousarray(Q.reshape(MOUT, 2, 128).transpose(2, 1, 0)).astype(np_dtype)
    return x3, r2t, qt


def unshard_output(ys):
    """Per-core [npc, 2, C, 2, 2, 16, 32] l-block parity planes -> [N, C, 64, 64]."""
    y5 = np.concatenate(ys, axis=0)  # [n, lb, C, top, left, vi_in, vj]
    n = y5.shape[0]
    # h = 32*lb + 2*vi_in + top ; w = 2*vj + left
    y = y5.transpose(0, 2, 1, 5, 3, 6, 4).reshape(n, C, 64, 64)
    return np.ascontiguousarray(y)


_NC_CACHE = {}


def kernel(x, Q, R):
    x3, r2t, qt = make_host_inputs(x, Q, R)
    n = x3.shape[0]
    assert n == N_CORES * N_PER_CORE
    if "nc" not in _NC_CACHE:
        _NC_CACHE["nc"] = build_nc()
    nc = _NC_CACHE["nc"]
    in_maps = [
        {
            "x": np.ascontiguousarray(x3[i * N_PER_CORE : (i + 1) * N_PER_CORE]),
            "r2t": r2t,
            "qt": qt,
        }
        for i in range(N_CORES)
    ]
    res = run_bass_kernel_spmd(nc, in_maps, list(range(N_CORES)))
    return unshard_output([res.results[i]["y"] for i in range(N_CORES)])


# revision 8
# speedup vs baseline: 1.0241x; 1.0241x over previous
# Trainium2 Bass kernel for nn_Conv2dSDK_QR: low-rank (Q @ R) factorized
# stride-1 3x3 conv expressed as two matmuls over 4x4/stride-2 windows.
#
# Math (per image, validated vs reference):
#   xp = zero-pad(x, 1)                              [128, 66, 66]
#   flatT[win*128+c, vi*32+vj] = xp[c, i+2vi, j+2vj] (win = i*4+j)
#   tT = R2 @ flatT                                  [256, 1024]
#   yT = Q @ tT                                      [512, 1024]
#   out[oc, 2vi+top, 2vj+left] = yT[(top*2+left)*128+oc, vi*32+vj]
# where R2 is R with columns permuted from (c*16+win) to (win*128+c)
# ordering, so each win-chunk of flatT is just a strided view of xp.
#
# Device layouts (host pre/post-processed so every PE stream and every DMA
# is contiguous):
#   space-to-depth: s2d[c, pi, pj, hi, wi] = xp[c, 2hi+pi, 2wi+pj] (66=2x33)
#   x3[lb][c, pi, pj, h, w] = s2d[c, pi, pj, 16*lb + h, w], h in [0,17)
#     (l-block chunks, boundary row hi=16 duplicated) -> window (i,j) of
#     l-block lb is the contiguous-inner view
#     x3[lb][:, i&1, j&1, (i>>1):(i>>1)+16, (j>>1):(j>>1)+32]
#   y per l-block: y3[lb][oc, top, left, vi_in, vj] = out[oc, 2(16lb+vi_in)+top, 2vj+left]
#
# Sharding: data-parallel over batch, 4 images per core across 8 cores.

import numpy as np

import concourse.bacc as bacc
import concourse.bass as bass
import concourse.mybir as mybir
import concourse.tile as tile
from concourse.bass_utils import run_bass_kernel_spmd

N_CORES = 8
N_PER_CORE = 4
C = 128          # channels (= partition dim)
H = W = 64
RANK = 256
MOUT = 512       # 4 placements * 128 out channels
NWIN = 16        # 4x4 window positions
DT = mybir.dt.float32
MM_DT = mybir.dt.float32r


def build_nc(n_per_core=N_PER_CORE, mm_dtype=MM_DT):
    nc = bacc.Bacc()
    x_ext = nc.declare_dram_parameter("x", [n_per_core, 2, C, 2, 2, 17, 33], mm_dtype, isOutput=False)
    r_ext = nc.declare_dram_parameter("r2t", [2, C, NWIN, 128], mm_dtype, isOutput=False)
    q_ext = nc.declare_dram_parameter("qt", [C, 2, MOUT], mm_dtype, isOutput=False)
    y_ext = nc.declare_dram_parameter("y", [n_per_core, 2, C, 2, 2, 16, 32], DT, isOutput=True)

    with tile.TileContext(nc) as tc:
        with (
            tc.tile_pool(name="weights", bufs=1) as wpool,
            tc.tile_pool(name="xp", bufs=4) as xpool,
            tc.tile_pool(name="tt", bufs=2) as tpool,
            tc.tile_pool(name="osb", bufs=4) as opool,
            tc.tile_pool(name="pt", bufs=4, space="PSUM") as ptpool,
            tc.tile_pool(name="py", bufs=4, space="PSUM") as pypool,
        ):
            # r2t[rc][c, win, r_in] = R2[rc*128+r_in, win*128+c]; rc0 first —
            # it gates the first matmul group.
            r2t = [wpool.tile([C, NWIN, 128], mm_dtype, tag=f"r2t{rc}", name=f"r2t{rc}") for rc in range(2)]
            # qt[r_in, rc, m] = Q[m, rc*128+r_in]  (lhsT chunks for matmul 2)
            qt = wpool.tile([C, 2, MOUT], mm_dtype)
            nc.scalar.dma_start(r2t[0][:], r_ext[0])
            nc.scalar.dma_start(r2t[1][:], r_ext[1])
            # qt rides the gpsimd SWDGE ring: it's only needed by the first
            # matmul-2 group (~25us in), and this keeps the scalar HWDGE ring
            # clear so r2t[1] lands before the (rc=1, lb=0) group needs it.
            nc.gpsimd.dma_start(qt[:], q_ext[:])

            for n in range(n_per_core):
                x3 = [xpool.tile([C, 2, 2, 17, 33], mm_dtype, tag="x3", name=f"x3_{n}_{lb}") for lb in range(2)]
                nc.sync.dma_start(x3[0][:], x_ext[n, 0])
                nc.sync.dma_start(x3[1][:], x_ext[n, 1])
                # tT[r_in, rc, vi_in, vj] per l-block
                tT = tpool.tile([C, 2, 2, 16, 32], mm_dtype)
                # rc-outer: the first two groups only need r2t[0], giving the
                # r2t[1] DMA until ~2 group-times after the first matmul.
                for rc in range(2):   # rank tiles of 128
                    for lb in range(2):   # l-blocks of 512 positions
                        pt = ptpool.tile([128, 16, 32], DT)
                        for win in range(NWIN):
                            i, j = divmod(win, 4)
                            rhs = x3[lb][:, i & 1, j & 1,
                                         (i >> 1) : (i >> 1) + 16,
                                         (j >> 1) : (j >> 1) + 32]
                            nc.tensor.matmul(
                                pt[:],
                                r2t[rc][:, win, :],
                                rhs,
                                start=(win == 0),
                                stop=(win == NWIN - 1),
                            )
                        nc.vector.tensor_copy(tT[:, rc, lb], pt[:])
                for lb in range(2):
                    osb = opool.tile([C, 2, 2, 16, 32], DT, tag="osb")
                    for mt in range(4):   # output row tiles: m = mt*128 + oc
                        py = pypool.tile([128, 16, 32], DT)
                        for rc in range(2):
                            nc.tensor.matmul(
                                py[:],
                                qt[:, rc, mt * 128 : (mt + 1) * 128],
                                tT[:, rc, lb],
                                start=(rc == 0),
                                stop=(rc == 1),
                            )
                        top, left = divmod(mt, 2)
                        nc.vector.tensor_copy(osb[:, top, left], py[:])
                    # stream output per (lb, top) half; the very last image's
                    # halves go out on both rings in parallel to cut the tail
                    # (the sync ring is long done with inputs by then).
                    for top in range(2):
                        eng = nc.sync if (n == n_per_core - 1 and top == 1) else nc.scalar
                        eng.dma_start(y_ext[n, lb, :, top], osb[:, top])
    nc.finalize()
    return nc


def make_host_inputs(x, Q, R, np_dtype=np.float32):
    """Full inputs -> (x3 chunks, r2t halves, qt) host arrays."""
    x = np.asarray(x, dtype=np.float32)
    Q = np.asarray(Q, dtype=np.float32)
    R = np.asarray(R, dtype=np.float32)
    n = x.shape[0]
    xpad = np.zeros((n, C, 66, 66), np.float32)
    xpad[:, :, 1 : 1 + H, 1 : 1 + W] = x
    # space-to-depth: s2d[n, c, pi, pj, hi, wi] = xpad[n, c, 2hi+pi, 2wi+pj]
    s2d = xpad.reshape(n, C, 33, 2, 33, 2).transpose(0, 1, 3, 5, 2, 4)
    # l-block chunks with duplicated boundary row hi=16:
    # x3[n, lb, c, pi, pj, h, w] = s2d[n, c, pi, pj, 16*lb+h, w]
    x3 = np.empty((n, 2, C, 2, 2, 17, 33), np.float32)
    x3[:, 0] = s2d[:, :, :, :, 0:17]
    x3[:, 1] = s2d[:, :, :, :, 16:33]
    x3 = np.ascontiguousarray(x3).astype(np_dtype)
    # permute R columns from (c*16+win) to (win*128+c), split by rank half
    R2 = R.reshape(RANK, C, NWIN).transpose(0, 2, 1).reshape(RANK, C * NWIN)
    r2t = R2.reshape(2, 128, NWIN, C).transpose(0, 3, 2, 1)  # [rc, c, win, r_in]
    r2t = np.ascontiguousarray(r2t).astype(np_dtype)
    qt = np.ascontiguousarray(Q.reshape(MOUT, 2, 128).transpose(2, 1, 0)).astype(np_dtype)
    return x3, r2t, qt


def unshard_output(ys):
    """Per-core [npc, 2, C, 2, 2, 16, 32] l-block parity planes -> [N, C, 64, 64]."""
    y5 = np.concatenate(ys, axis=0)  # [n, lb, C, top, left, vi_in, vj]
    n = y5.shape[0]
    # h = 32*lb + 2*vi_in + top ; w = 2*vj + left
    y = y5.transpose(0, 2, 1, 5, 3, 6, 4).reshape(n, C, 64, 64)
    return np.ascontiguousarray(y)


_NC_CACHE = {}


def kernel(x, Q, R):
    x3, r2t, qt = make_host_inputs(x, Q, R)
    n = x3.shape[0]
    assert n == N_CORES * N_PER_CORE
    if "nc" not in _NC_CACHE:
        _NC_CACHE["nc"] = build_nc()
    nc = _NC_CACHE["nc"]
    in_maps = [
        {
            "x": np.ascontiguousarray(x3[i * N_PER_CORE : (i + 1) * N_PER_CORE]),
            "r2t": r2t,
            "qt": qt,
        }
        for i in range(N_CORES)
    ]
    res = run_bass_kernel_spmd(nc, in_maps, list(range(N_CORES)))
    return unshard_output([res.results[i]["y"] for i in range(N_CORES)])


# revision 13
# speedup vs baseline: 1.0597x; 1.0348x over previous
# Trainium2 Bass kernel for nn_Conv2dSDK_QR: low-rank (Q @ R) factorized
# stride-1 3x3 conv expressed as two matmuls over 4x4/stride-2 windows.
#
# Math (per image, validated vs reference):
#   xp = zero-pad(x, 1)                              [128, 66, 66]
#   flatT[win*128+c, vi*32+vj] = xp[c, i+2vi, j+2vj] (win = i*4+j)
#   tT = R2 @ flatT                                  [256, 1024]
#   yT = Q @ tT                                      [512, 1024]
#   out[oc, 2vi+top, 2vj+left] = yT[(top*2+left)*128+oc, vi*32+vj]
# where R2 is R with columns permuted from (c*16+win) to (win*128+c)
# ordering, so each win-chunk of flatT is just a strided view of xp.
#
# Device layouts (host pre/post-processed so every PE stream and every DMA
# is contiguous):
#   space-to-depth: s2d[c, pi, pj, hi, wi] = xp[c, 2hi+pi, 2wi+pj] (66=2x33)
#   x3[lb][c, pi, pj, h, w] = s2d[c, pi, pj, 16*lb + h, w], h in [0,17)
#     (l-block chunks, boundary row hi=16 duplicated) -> window (i,j) of
#     l-block lb is the contiguous-inner view
#     x3[lb][:, i&1, j&1, (i>>1):(i>>1)+16, (j>>1):(j>>1)+32]
#   y per l-block: y3[lb][oc, top, left, vi_in, vj] = out[oc, 2(16lb+vi_in)+top, 2vj+left]
#
# Sharding: data-parallel over batch, 4 images per core across 8 cores.

import numpy as np

import concourse.bacc as bacc
import concourse.bass as bass
import concourse.mybir as mybir
import concourse.tile as tile
from concourse.bass_utils import run_bass_kernel_spmd

N_CORES = 8
N_PER_CORE = 4
C = 128          # channels (= partition dim)
H = W = 64
RANK = 256
MOUT = 512       # 4 placements * 128 out channels
NWIN = 16        # 4x4 window positions
DT = mybir.dt.float32
MM_DT = mybir.dt.float32r
# Window processing order: even-i (pi=0) windows first, so the first matmul
# group can start as soon as the pi=0 half of the image chunk and the first
# weight chunk have landed. The r2t win axis is host-permuted to this order.
WIN_SEQ = [0, 1, 2, 3, 8, 9, 10, 11, 4, 5, 6, 7, 12, 13, 14, 15]


def build_nc(n_per_core=N_PER_CORE, mm_dtype=MM_DT):
    nc = bacc.Bacc()
    x_ext = nc.declare_dram_parameter("x", [n_per_core, 2, C, 2, 2, 17, 33], mm_dtype, isOutput=False)
    r_ext = nc.declare_dram_parameter("r2t", [2, C, NWIN, 128], mm_dtype, isOutput=False)
    q_ext = nc.declare_dram_parameter("qt", [C, 2, MOUT], mm_dtype, isOutput=False)
    y_ext = nc.declare_dram_parameter("y", [n_per_core, 2, C, 2, 2, 16, 32], DT, isOutput=True)

    with tile.TileContext(nc) as tc:
        with (
            tc.tile_pool(name="weights", bufs=1) as wpool,
            tc.tile_pool(name="xp", bufs=4) as xpool,
            tc.tile_pool(name="tt", bufs=2) as tpool,
            tc.tile_pool(name="osb", bufs=4) as opool,
            tc.tile_pool(name="pt", bufs=4, space="PSUM") as ptpool,
            tc.tile_pool(name="py", bufs=4, space="PSUM") as pypool,
        ):
            # r2t[rc][c, win, r_in] = R2[rc*128+r_in, win*128+c]; rc0 first —
            # it gates the first matmul group.
            r2t = [wpool.tile([C, NWIN, 128], mm_dtype, tag=f"r2t{rc}", name=f"r2t{rc}") for rc in range(2)]
            # qt[r_in, rc, m] = Q[m, rc*128+r_in]  (lhsT chunks for matmul 2)
            qt = wpool.tile([C, 2, MOUT], mm_dtype)
            # r2t[0] in two chunks so the first windows' weights land early.
            nc.scalar.dma_start(r2t[0][:, :8], r_ext[0][:, :8])
            nc.scalar.dma_start(r2t[0][:, 8:], r_ext[0][:, 8:])
            nc.scalar.dma_start(r2t[1][:], r_ext[1])
            # qt rides the gpsimd SWDGE ring: it's only needed by the first
            # matmul-2 group (~25us in), and this keeps the scalar HWDGE ring
            # clear so r2t[1] lands before the (rc=1, lb=0) group needs it.
            nc.gpsimd.dma_start(qt[:], q_ext[:])

            for n in range(n_per_core):
                x3 = [xpool.tile([C, 2, 2, 17, 33], mm_dtype, tag="x3", name=f"x3_{n}_{lb}") for lb in range(2)]
                for lb in range(2):
                    # pi-split halves: the even-i windows only read pi=0.
                    nc.sync.dma_start(x3[lb][:, 0], x_ext[n, lb, :, 0])
                    nc.sync.dma_start(x3[lb][:, 1], x_ext[n, lb, :, 1])
                # tT[r_in, rc, vi_in, vj] per l-block
                tT = tpool.tile([C, 2, 2, 16, 32], mm_dtype)
                # rc-outer: the first two groups only need r2t[0], giving the
                # r2t[1] DMA until ~2 group-times after the first matmul.
                for rc in range(2):   # rank tiles of 128
                    for lb in range(2):   # l-blocks of 512 positions
                        pt = ptpool.tile([128, 16, 32], DT)
                        for k, win in enumerate(WIN_SEQ):
                            i, j = divmod(win, 4)
                            rhs = x3[lb][:, i & 1, j & 1,
                                         (i >> 1) : (i >> 1) + 16,
                                         (j >> 1) : (j >> 1) + 32]
                            nc.tensor.matmul(
                                pt[:],
                                r2t[rc][:, k, :],
                                rhs,
                                start=(k == 0),
                                stop=(k == NWIN - 1),
                            )
                        nc.vector.tensor_copy(tT[:, rc, lb], pt[:])
                for lb in range(2):
                    osb = opool.tile([C, 2, 2, 16, 32], DT, tag="osb")
                    for mt in range(4):   # output row tiles: m = mt*128 + oc
                        py = pypool.tile([128, 16, 32], DT)
                        for rc in range(2):
                            nc.tensor.matmul(
                                py[:],
                                qt[:, rc, mt * 128 : (mt + 1) * 128],
                                tT[:, rc, lb],
                                start=(rc == 0),
                                stop=(rc == 1),
                            )
                        top, left = divmod(mt, 2)
                        nc.vector.tensor_copy(osb[:, top, left], py[:])
                    # stream output per (lb, top) half; the very last image's
                    # output goes out in quarters alternating across both
                    # rings to cut the exposed tail (the sync ring is long
                    # done with inputs by then).
                    if n == n_per_core - 1:
                        for q, (top, left) in enumerate([(0, 0), (0, 1), (1, 0), (1, 1)]):
                            eng = nc.sync if q % 2 else nc.scalar
                            eng.dma_start(y_ext[n, lb, :, top, left], osb[:, top, left])
                    else:
                        for top in range(2):
                            nc.scalar.dma_start(y_ext[n, lb, :, top], osb[:, top])
    nc.finalize()
    return nc


def make_host_inputs(x, Q, R, np_dtype=np.float32):
    """Full inputs -> (x3 chunks, r2t halves, qt) host arrays."""
    x = np.asarray(x, dtype=np.float32)
    Q = np.asarray(Q, dtype=np.float32)
    R = np.asarray(R, dtype=np.float32)
    n = x.shape[0]
    xpad = np.zeros((n, C, 66, 66), np.float32)
    xpad[:, :, 1 : 1 + H, 1 : 1 + W] = x
    # space-to-depth: s2d[n, c, pi, pj, hi, wi] = xpad[n, c, 2hi+pi, 2wi+pj]
    s2d = xpad.reshape(n, C, 33, 2, 33, 2).transpose(0, 1, 3, 5, 2, 4)
    # l-block chunks with duplicated boundary row hi=16:
    # x3[n, lb, c, pi, pj, h, w] = s2d[n, c, pi, pj, 16*lb+h, w]
    x3 = np.empty((n, 2, C, 2, 2, 17, 33), np.float32)
    x3[:, 0] = s2d[:, :, :, :, 0:17]
    x3[:, 1] = s2d[:, :, :, :, 16:33]
    x3 = np.ascontiguousarray(x3).astype(np_dtype)
    # permute R columns from (c*16+win) to (win*128+c), split by rank half
    R2 = R.reshape(RANK, C, NWIN).transpose(0, 2, 1).reshape(RANK, C * NWIN)
    r2t = R2.reshape(2, 128, NWIN, C).transpose(0, 3, 2, 1)  # [rc, c, win, r_in]
    r2t = r2t[:, :, WIN_SEQ, :]  # win axis in device processing order
    r2t = np.ascontiguousarray(r2t).astype(np_dtype)
    qt = np.ascontiguousarray(Q.reshape(MOUT, 2, 128).transpose(2, 1, 0)).astype(np_dtype)
    return x3, r2t, qt


def unshard_output(ys):
    """Per-core [npc, 2, C, 2, 2, 16, 32] l-block parity planes -> [N, C, 64, 64]."""
    y5 = np.concatenate(ys, axis=0)  # [n, lb, C, top, left, vi_in, vj]
    n = y5.shape[0]
    # h = 32*lb + 2*vi_in + top ; w = 2*vj + left
    y = y5.transpose(0, 2, 1, 5, 3, 6, 4).reshape(n, C, 64, 64)
    return np.ascontiguousarray(y)


_NC_CACHE = {}


def kernel(x, Q, R):
    x3, r2t, qt = make_host_inputs(x, Q, R)
    n = x3.shape[0]
    assert n == N_CORES * N_PER_CORE
    if "nc" not in _NC_CACHE:
        _NC_CACHE["nc"] = build_nc()
    nc = _NC_CACHE["nc"]
    in_maps = [
        {
            "x": np.ascontiguousarray(x3[i * N_PER_CORE : (i + 1) * N_PER_CORE]),
            "r2t": r2t,
            "qt": qt,
        }
        for i in range(N_CORES)
    ]
    res = run_bass_kernel_spmd(nc, in_maps, list(range(N_CORES)))
    return unshard_output([res.results[i]["y"] for i in range(N_CORES)])
